# revision 4
# baseline (speedup 1.0000x reference)
"""Single-head attention (B=8, N=2048, E=1024) on 8 TRN2 NeuronCores.

Sharding: data-parallel over batch - core i computes batch element i fully.

Weight-fusion restructuring: softmax(q k^T) depends on the weights only
through M = Wq^T Wk (plus a per-key bias correction), so M is folded on
the host (one E^3 sgemm of weight prep, shared by all batches/cores) and
the device computes
  uT[e2,i] = sum_e1 M[e1,e2] x[i,e1]          (256 matmuls)
  sT[j,i]  = sum_e2 x[j,e2] uT[e2,i]          (512)
instead of q-proj + k-proj + scores (512+512). Bias handling is exact:
  (q_i+bq).(k_j+bk) = x_i M x_j + [row-const terms that cancel in
  softmax] + c_j,  c_j = x_j.(Wk^T bq)
c_j is folded into the exp's per-partition bias on the host too.

Per-core dataflow (all matmul compute bf16, f32 PSUM):
  v    ->  vt[n(part), e] + bv         (DVE add drain)
  uT   ->  uTck[e2(part), i]           (ACT identity drain)
  expT[j(part), i] = exp(SCALE*sT + cb[j])   (ACT exp drain)
  sacc[j(part), i] += expT  per jt     (DVE running sum over j-tiles)
  denom: ones-matmul over j-partitions of sacc (after numerator it=0,
         so the DVE sum chain is off the PE critical path)
  out[i,e] = (expT.T @ v) * (1/denom)  (ACT copy-scale, DMA out)

Softmax skips max-subtraction: scores ~N(0,1), max |s| < ~15, exp fits
f32/bf16 range fine and softmax is shift-invariant.
"""

import numpy as np
import ml_dtypes

P = 128
E = 1024
N = 2048
KO = E // P      # 8 contraction subtiles
NT = N // P      # 16 row tiles
NCH = N // 512   # 4 chunks of 512
SCALE = 0.03125  # 1/sqrt(1024)
NWARM = 10

_CACHE = {}


def _build():
    import concourse.bacc as bacc
    import concourse.tile as tile
    import concourse.mybir as mybir

    f32 = mybir.dt.float32
    bf16 = mybir.dt.bfloat16
    AF = mybir.ActivationFunctionType
    Alu = mybir.AluOpType

    nc = bacc.Bacc("TRN2", target_bir_lowering=False, debug=False, num_devices=8)
    xT_d = nc.dram_tensor("xT", [E, N], bf16, kind="ExternalInput")
    m_d = nc.dram_tensor("m", [E, E], bf16, kind="ExternalInput")     # M[e1, e2]
    wv_d = nc.dram_tensor("wv", [E, E], bf16, kind="ExternalInput")   # [e_in, e_out] (= Wv^T)
    cb_d = nc.dram_tensor("cb", [P, 16], f32, kind="ExternalInput")
    bv_d = nc.dram_tensor("bv", [P, E], f32, kind="ExternalInput")
    out_d = nc.dram_tensor("out", [N, E], bf16, kind="ExternalOutput")

    xT_r = xT_d.ap().rearrange("(ko p) (c n) -> c p ko n", p=P, n=512)
    m_r = m_d.ap().rearrange("(ko p) e -> ko p e", p=P)
    wv_r = wv_d.ap().rearrange("(ko p) e -> ko p e", p=P)
    out_r = out_d.ap().rearrange("(it p) e -> it p e", p=P)

    with tile.TileContext(nc) as tc:
        with (
            tc.tile_pool(name="const", bufs=1) as const,
            tc.tile_pool(name="big", bufs=1) as big,
        ):
            cb_t = const.tile([P, 16], f32, tag="cb")
            nc.gpsimd.dma_start(cb_t[:], cb_d.ap())
            bv_t = const.tile([P, E], f32, tag="bv")
            nc.gpsimd.dma_start(bv_t[:], bv_d.ap())
            ones_t = const.tile([P, 1], bf16, tag="ones")
            nc.vector.memset(ones_t[:], 1.0)

            # persistent SBUF tensors
            xck = [[None] * NCH for _ in range(KO)]  # x^T [e(part), n]
            for k in range(KO):
                for c in range(NCH):
                    xck[k][c] = big.tile([P, 512], bf16, tag=f"x{k}_{c}",
                                         name=f"x{k}_{c}")
            uTc = [big.tile([P, KO, 512], bf16, tag=f"uT{c}", name=f"uT{c}")
                   for c in range(NCH)]
            vt = big.tile([P, NT, E], bf16, tag="v")

            with (
                tc.tile_pool(name="pin", bufs=1) as pin,
                tc.tile_pool(name="pps1", bufs=8, space="PSUM") as pps1,
            ):
                wvt = [pin.tile([P, E], bf16, tag=f"wv{k}", name=f"wv{k}")
                       for k in range(KO)]
                mk = [pin.tile([P, E], bf16, tag=f"m{k}", name=f"m{k}")
                      for k in range(KO)]

                # DMA order = consumption order across 3 rings (~150GB/s
                # each while <=2 stream concurrently):
                #   ACT: xck c0, c1      (v-proj lhsT for nt 0..7)
                #   SP:  wv, xck c2, c3  (v-proj rhs first - it paces nt0)
                #   POOL: cb, bv, M      (M only needed at uT, ~35us slack)
                for k in range(KO):
                    nc.scalar.dma_start(xck[k][0][:], xT_r[0][:, k, :])
                for k in range(KO):
                    nc.sync.dma_start(wvt[k][:], wv_r[k])
                for k in range(KO):
                    nc.scalar.dma_start(xck[k][1][:], xT_r[1][:, k, :])
                for c in (2, 3):
                    for k in range(KO):
                        nc.sync.dma_start(xck[k][c][:], xT_r[c][:, k, :])
                # m rides the ACT ring behind xck c0/c1: it is only needed
                # at uT (~75us in), and streaming it on the POOL ring during
                # the v-proj head window would steal HBM bandwidth from the
                # (xck c0, wv) pair stream that paces the first nt-block.
                for k in range(KO):
                    nc.scalar.dma_start(mk[k][:], m_r[k])

                # PE warmup: bridge the engine preamble until the first
                # (xck c0, wv) tiles land; keeps the clock ramp going.
                scratch = pin.tile([P, 512], bf16, tag="warm_in")
                nc.vector.memset(scratch[:], 0.0)
                junk_ps = None
                for _ in range(NWARM):
                    junk_ps = pps1.tile([P, 512], f32, tag="ps", name="ps_w")
                    nc.tensor.matmul(
                        junk_ps[:], lhsT=scratch[:, :P], rhs=scratch[:],
                        start=True, stop=True,
                    )
                junk_sb = pin.tile([P, 1], f32, tag="warm_out")
                nc.vector.tensor_copy(junk_sb[:], junk_ps[:, 0:1])
                junk_d = nc.dram_tensor("warm_scratch", [P, 1], f32, kind="Internal")
                nc.sync.dma_start(junk_d.ap(), junk_sb[:])

                # ---- v = x Wv^T + bv  [n(part), e]  (DVE drain) ----
                # First nt-block is k-OUTER (8 live banks): it consumes each
                # (xck c0, wv) k-slice pair right as the two DMA rings land
                # it, so v-proj streams behind the input DMA instead of
                # waiting ~14us for all of wv.
                psb = [pps1.tile([P, 512], f32, tag="ps", name=f"psv{_i}")
                       for _i in range(8)]
                for k in range(KO):
                    for nt in range(4):
                        for ech in range(2):
                            nc.tensor.matmul(
                                psb[nt * 2 + ech][:],
                                lhsT=xck[k][0][:, nt * P:(nt + 1) * P],
                                rhs=wvt[k][:, ech * 512:(ech + 1) * 512],
                                start=(k == 0),
                                stop=(k == KO - 1),
                            )
                for nt in range(4):
                    for ech in range(2):
                        esl = slice(ech * 512, (ech + 1) * 512)
                        nc.vector.tensor_tensor(
                            out=vt[:, nt, esl], in0=psb[nt * 2 + ech][:],
                            in1=bv_t[:, esl], op=Alu.add,
                        )
                for nt in range(4, NT):
                    ps = [pps1.tile([P, 512], f32, tag="ps", name=f"ps{_i}") for _i in range(2)]
                    for k in range(KO):
                        for ech in range(2):
                            nc.tensor.matmul(
                                ps[ech][:],
                                lhsT=xck[k][nt // 4][:, (nt % 4) * P:(nt % 4 + 1) * P],
                                rhs=wvt[k][:, ech * 512:(ech + 1) * 512],
                                start=(k == 0),
                                stop=(k == KO - 1),
                            )
                    for ech in range(2):
                        esl = slice(ech * 512, (ech + 1) * 512)
                        nc.vector.tensor_tensor(
                            out=vt[:, nt, esl], in0=ps[ech][:], in1=bv_t[:, esl],
                            op=Alu.add,
                        )

                # ---- uT = M^T x^T  [e2(part), i]  (ACT drain) ----
                for e2t in range(KO):
                    ps = [pps1.tile([P, 512], f32, tag="ps", name=f"ps{_i}") for _i in range(NCH)]
                    for k in range(KO):
                        for c in range(NCH):
                            nc.tensor.matmul(
                                ps[c][:],
                                lhsT=mk[k][:, e2t * P:(e2t + 1) * P],
                                rhs=xck[k][c][:],
                                start=(k == 0),
                                stop=(k == KO - 1),
                            )
                    for c in range(NCH):
                        nc.scalar.activation(
                            uTc[c][:, e2t, :], ps[c][:], AF.Identity, scale=1.0,
                        )

            with tc.tile_pool(name="attn", bufs=1) as attn:
                expT = [attn.tile([P, N], bf16, tag=f"expT{jt}", name=f"expT{jt}")
                        for jt in range(NT)]
                sacc = attn.tile([P, N], f32, tag="sacc")
                sume_bf = attn.tile([P, N], bf16, tag="sume_bf")
                rdent = attn.tile([P, 16], f32, tag="rdent")

                # ---- scoresT[j,i] = x M x^T, exp on ACT, running row-sums
                # over j-tiles on DVE ----
                with tc.tile_pool(name="psc", bufs=8, space="PSUM") as psc:
                    for jt in range(NT):
                        ps = [psc.tile([P, 512], f32, tag="ps_s", name=f"pss{_i}") for _i in range(NCH)]
                        for k in range(KO):
                            for c in range(NCH):
                                nc.tensor.matmul(
                                    ps[c][:],
                                    lhsT=xck[k][jt // 4][:, (jt % 4) * P:(jt % 4 + 1) * P],
                                    rhs=uTc[c][:, k, :],
                                    start=(k == 0),
                                    stop=(k == KO - 1),
                                )
                        for c in range(NCH):
                            nc.scalar.activation(
                                expT[jt][:, c * 512:(c + 1) * 512], ps[c][:],
                                AF.Exp, bias=cb_t[:, jt:jt + 1], scale=SCALE,
                            )
                        if jt == 0:
                            nc.vector.tensor_copy(sacc[:], expT[0][:])
                        else:
                            nc.vector.tensor_tensor(
                                out=sacc[:], in0=sacc[:], in1=expT[jt][:], op=Alu.add,
                            )

                # ---- numerator + scale + store ----
                # Denominator matmuls are emitted after it=0's numerator
                # group: they depend on the DVE sum chain (exp jt=15 ->
                # sacc -> sume_bf) which finishes ~5us after the last
                # scores matmul; emitting them first would stall the PE.
                with tc.tile_pool(name="pnum", bufs=4, space="PSUM") as pnum:
                    nc.vector.tensor_copy(sume_bf[:], sacc[:])
                    for it in range(NT):
                        ps = [pnum.tile([P, 512], f32, tag="ps_n", name=f"psn{_i}") for _i in range(2)]
                        for jt in range(NT):
                            for ech in range(2):
                                nc.tensor.matmul(
                                    ps[ech][:],
                                    lhsT=expT[jt][:, it * P:(it + 1) * P],
                                    rhs=vt[:, jt, ech * 512:(ech + 1) * 512],
                                    start=(jt == 0),
                                    stop=(jt == NT - 1),
                                )
                        if it == 0:
                            pd = pnum.tile([P, 16], f32, tag="pd", bufs=1)
                            for dt in range(NT):
                                nc.tensor.matmul(
                                    pd[:, dt:dt + 1],
                                    lhsT=sume_bf[:, dt * P:(dt + 1) * P],
                                    rhs=ones_t[:],
                                    start=True, stop=True,
                                )
                            nc.vector.reciprocal(rdent[:], pd[:])
                        # output staged bf16 (halves the out-DMA); the two
                        # halves drain on ACT and DVE in parallel
                        osb = attn.tile([P, E], bf16, tag="osb", bufs=3)
                        nc.scalar.activation(
                            osb[:, 0:512], ps[0][:], AF.Copy,
                            scale=rdent[:, it:it + 1],
                        )
                        nc.vector.tensor_scalar_mul(
                            osb[:, 512:E], ps[1][:], rdent[:, it:it + 1],
                        )
                        for ech in range(2):
                            esl = slice(ech * 512, (ech + 1) * 512)
                            nc.sync.dma_start(out_r[it][:, esl], osb[:, esl])
    nc.compile()
    return nc


def get_nc():
    if "nc" not in _CACHE:
        _CACHE["nc"] = _build()
    return _CACHE["nc"]


def prepare_in_maps(x, W_qkv, b_qkv):
    bf = ml_dtypes.bfloat16
    x = np.asarray(x, dtype=np.float32)
    W = np.asarray(W_qkv, dtype=np.float32)
    b = np.asarray(b_qkv, dtype=np.float32)
    assert x.shape == (8, N, E) and W.shape == (3 * E, E) and b.shape == (3 * E,)
    xT = np.ascontiguousarray(np.transpose(x, (0, 2, 1))).astype(bf)  # [8, E, N]
    # fused QK weight: scores depend on Wq, Wk only through M = Wq^T Wk
    m = np.ascontiguousarray(W[:E].T @ W[E:2 * E]).astype(bf)         # [e1, e2]
    wv = np.ascontiguousarray(W[2 * E:].T).astype(bf)                 # [e_in, e_out]
    bv = np.ascontiguousarray(np.broadcast_to(b[2 * E:], (P, E)))     # [P, E]
    # per-key score bias c_j = x_j . (Wk^T bq), folded into exp bias
    m1 = W[E:2 * E].T @ b[:E]                                         # [E]
    cb = SCALE * (x @ m1)                                             # [8, N]
    cb = np.ascontiguousarray(cb.reshape(8, 16, P).transpose(0, 2, 1)).astype(np.float32)
    return [{"xT": xT[i], "m": m, "wv": wv,
             "cb": cb[i], "bv": bv} for i in range(8)]


def kernel(x, W_qkv, b_qkv):
    from concourse.bass_utils import run_bass_kernel_spmd

    nc = get_nc()
    in_maps = prepare_in_maps(x, W_qkv, b_qkv)
    res = run_bass_kernel_spmd(nc, in_maps, core_ids=list(range(8)))
    return np.stack([res.results[i]["out"] for i in range(8)], axis=0).astype(np.float32)


# revision 5
# speedup vs baseline: 1.1883x; 1.1883x over previous
"""Single-head attention (B=8, N=2048, E=1024) on 8 TRN2 NeuronCores.

Sharding: data-parallel over batch - core i computes batch element i fully.

Weight-fusion restructuring: softmax(q k^T) depends on the weights only
through M = Wq^T Wk (plus a per-key bias correction), so M is folded on
the host (one E^3 sgemm of weight prep, shared by all batches/cores) and
the device computes
  uT[e2,i] = sum_e1 M[e1,e2] x[i,e1]          (256 matmuls)
  sT[j,i]  = sum_e2 x[j,e2] uT[e2,i]          (512)
instead of q-proj + k-proj + scores (512+512). Bias handling is exact:
  (q_i+bq).(k_j+bk) = x_i M x_j + [row-const terms that cancel in
  softmax] + c_j,  c_j = x_j.(Wk^T bq)
c_j is folded into the exp's per-partition bias on the host too.

Per-core dataflow (all matmul compute bf16, f32 PSUM):
  v    ->  vt[n(part), e] + bv         (DVE add drain)
  uT   ->  uTck[e2(part), i]           (ACT identity drain)
  expT[j(part), i] = exp(SCALE*sT + cb[j])   (ACT exp drain)
  sacc[j(part), i] += expT  per jt     (DVE running sum over j-tiles)
  denom: ones-matmul over j-partitions of sacc (after numerator it=0,
         so the DVE sum chain is off the PE critical path)
  out[i,e] = (expT.T @ v) * (1/denom)  (ACT copy-scale, DMA out)

Softmax skips max-subtraction: scores ~N(0,1), max |s| < ~15, exp fits
f32/bf16 range fine and softmax is shift-invariant.
"""

import numpy as np
import ml_dtypes

P = 128
E = 1024
N = 2048
KO = E // P      # 8 contraction subtiles
NT = N // P      # 16 row tiles
NCH = N // 512   # 4 chunks of 512
SCALE = 0.03125  # 1/sqrt(1024)
NWARM = 6

_CACHE = {}


def _build():
    import concourse.bacc as bacc
    import concourse.tile as tile
    import concourse.mybir as mybir

    f32 = mybir.dt.float32
    bf16 = mybir.dt.bfloat16
    AF = mybir.ActivationFunctionType
    Alu = mybir.AluOpType

    nc = bacc.Bacc("TRN2", target_bir_lowering=False, debug=False, num_devices=8)
    xT_d = nc.dram_tensor("xT", [E, N], bf16, kind="ExternalInput")
    m_d = nc.dram_tensor("m", [E, E], bf16, kind="ExternalInput")     # M[e1, e2]
    wv_d = nc.dram_tensor("wv", [E, E], bf16, kind="ExternalInput")   # [e_in, e_out] (= Wv^T)
    cb_d = nc.dram_tensor("cb", [P, 16], f32, kind="ExternalInput")
    bv_d = nc.dram_tensor("bv", [P, E], f32, kind="ExternalInput")
    out_d = nc.dram_tensor("out", [N, E], f32, kind="ExternalOutput")

    xT_r = xT_d.ap().rearrange("(ko p) (c n) -> c p ko n", p=P, n=512)
    m_r = m_d.ap().rearrange("(ko p) e -> ko p e", p=P)
    wv_r = wv_d.ap().rearrange("(ko p) e -> ko p e", p=P)
    out_r = out_d.ap().rearrange("(it p) e -> it p e", p=P)

    with tile.TileContext(nc) as tc:
        with (
            tc.tile_pool(name="const", bufs=1) as const,
            tc.tile_pool(name="big", bufs=1) as big,
        ):
            cb_t = const.tile([P, 16], f32, tag="cb")
            nc.gpsimd.dma_start(cb_t[:], cb_d.ap())
            bv_t = const.tile([P, E], f32, tag="bv")
            nc.gpsimd.dma_start(bv_t[:], bv_d.ap())
            ones_t = const.tile([P, 1], bf16, tag="ones")
            nc.vector.memset(ones_t[:], 1.0)

            # persistent SBUF tensors
            xck = [[None] * NCH for _ in range(KO)]  # x^T [e(part), n]
            for k in range(KO):
                for c in range(NCH):
                    xck[k][c] = big.tile([P, 512], bf16, tag=f"x{k}_{c}",
                                         name=f"x{k}_{c}")
            uTc = [big.tile([P, KO, 512], bf16, tag=f"uT{c}", name=f"uT{c}")
                   for c in range(NCH)]
            vt = big.tile([P, NT, E], bf16, tag="v")

            with (
                tc.tile_pool(name="pin", bufs=1) as pin,
                tc.tile_pool(name="pps1", bufs=8, space="PSUM") as pps1,
            ):
                wvt = [pin.tile([P, E], bf16, tag=f"wv{k}", name=f"wv{k}")
                       for k in range(KO)]
                mk = [pin.tile([P, E], bf16, tag=f"m{k}", name=f"m{k}")
                      for k in range(KO)]

                # DMA order = consumption order across 3 rings (~150GB/s
                # each while <=2 stream concurrently):
                #   ACT: xck c0, c1      (v-proj lhsT for nt 0..7)
                #   SP:  wv, xck c2, c3  (v-proj rhs first - it paces nt0)
                #   POOL: cb, bv, M      (M only needed at uT, ~35us slack)
                for k in range(KO):
                    nc.scalar.dma_start(xck[k][0][:], xT_r[0][:, k, :])
                for k in range(KO):
                    nc.sync.dma_start(wvt[k][:], wv_r[k])
                for k in range(KO):
                    nc.scalar.dma_start(xck[k][1][:], xT_r[1][:, k, :])
                for c in (2, 3):
                    for k in range(KO):
                        nc.sync.dma_start(xck[k][c][:], xT_r[c][:, k, :])
                # m rides the ACT ring behind xck c0/c1: it is only needed
                # at uT (~75us in), and streaming it on the POOL ring during
                # the v-proj head window would steal HBM bandwidth from the
                # (xck c0, wv) pair stream that paces the first nt-block.
                for k in range(KO):
                    nc.scalar.dma_start(mk[k][:], m_r[k])

                # PE warmup: bridge the engine preamble until the first
                # (xck c0, wv) tiles land; keeps the clock ramp going.
                scratch = pin.tile([P, 512], bf16, tag="warm_in")
                nc.vector.memset(scratch[:], 0.0)
                junk_ps = None
                for _ in range(NWARM):
                    junk_ps = pps1.tile([P, 512], f32, tag="ps", name="ps_w")
                    nc.tensor.matmul(
                        junk_ps[:], lhsT=scratch[:, :P], rhs=scratch[:],
                        start=True, stop=True,
                    )
                junk_sb = pin.tile([P, 1], f32, tag="warm_out")
                nc.vector.tensor_copy(junk_sb[:], junk_ps[:, 0:1])
                junk_d = nc.dram_tensor("warm_scratch", [P, 1], f32, kind="Internal")
                nc.sync.dma_start(junk_d.ap(), junk_sb[:])

                # ---- v = x Wv^T + bv  [n(part), e]  (DVE drain) ----
                # First nt-block is k-OUTER (8 live banks): it consumes each
                # (xck c0, wv) k-slice pair right as the two DMA rings land
                # it, so v-proj streams behind the input DMA instead of
                # waiting ~14us for all of wv.
                psb = [pps1.tile([P, 512], f32, tag="ps", name=f"psv{_i}")
                       for _i in range(8)]
                for k in range(KO):
                    for nt in range(4):
                        for ech in range(2):
                            nc.tensor.matmul(
                                psb[nt * 2 + ech][:],
                                lhsT=xck[k][0][:, nt * P:(nt + 1) * P],
                                rhs=wvt[k][:, ech * 512:(ech + 1) * 512],
                                start=(k == 0),
                                stop=(k == KO - 1),
                            )
                for nt in range(4):
                    for ech in range(2):
                        esl = slice(ech * 512, (ech + 1) * 512)
                        nc.vector.tensor_tensor(
                            out=vt[:, nt, esl], in0=psb[nt * 2 + ech][:],
                            in1=bv_t[:, esl], op=Alu.add,
                        )
                for nt in range(4, NT):
                    ps = [pps1.tile([P, 512], f32, tag="ps", name=f"ps{_i}") for _i in range(2)]
                    for k in range(KO):
                        for ech in range(2):
                            nc.tensor.matmul(
                                ps[ech][:],
                                lhsT=xck[k][nt // 4][:, (nt % 4) * P:(nt % 4 + 1) * P],
                                rhs=wvt[k][:, ech * 512:(ech + 1) * 512],
                                start=(k == 0),
                                stop=(k == KO - 1),
                            )
                    for ech in range(2):
                        esl = slice(ech * 512, (ech + 1) * 512)
                        nc.vector.tensor_tensor(
                            out=vt[:, nt, esl], in0=ps[ech][:], in1=bv_t[:, esl],
                            op=Alu.add,
                        )

                # ---- uT = M^T x^T  [e2(part), i]  (ACT drain) ----
                for e2t in range(KO):
                    ps = [pps1.tile([P, 512], f32, tag="ps", name=f"ps{_i}") for _i in range(NCH)]
                    for k in range(KO):
                        for c in range(NCH):
                            nc.tensor.matmul(
                                ps[c][:],
                                lhsT=mk[k][:, e2t * P:(e2t + 1) * P],
                                rhs=xck[k][c][:],
                                start=(k == 0),
                                stop=(k == KO - 1),
                            )
                    for c in range(NCH):
                        nc.scalar.activation(
                            uTc[c][:, e2t, :], ps[c][:], AF.Identity, scale=1.0,
                        )

            with tc.tile_pool(name="attn", bufs=1) as attn:
                expT = [attn.tile([P, N], bf16, tag=f"expT{jt}", name=f"expT{jt}")
                        for jt in range(NT)]
                sacc = attn.tile([P, N], f32, tag="sacc")
                sume_bf = attn.tile([P, N], bf16, tag="sume_bf")
                rdent = attn.tile([P, 16], f32, tag="rdent")

                # ---- scoresT[j,i] = x M x^T, exp on ACT, running row-sums
                # over j-tiles on DVE ----
                with tc.tile_pool(name="psc", bufs=8, space="PSUM") as psc:
                    for jt in range(NT):
                        ps = [psc.tile([P, 512], f32, tag="ps_s", name=f"pss{_i}") for _i in range(NCH)]
                        for k in range(KO):
                            for c in range(NCH):
                                nc.tensor.matmul(
                                    ps[c][:],
                                    lhsT=xck[k][jt // 4][:, (jt % 4) * P:(jt % 4 + 1) * P],
                                    rhs=uTc[c][:, k, :],
                                    start=(k == 0),
                                    stop=(k == KO - 1),
                                )
                        for c in range(NCH):
                            nc.scalar.activation(
                                expT[jt][:, c * 512:(c + 1) * 512], ps[c][:],
                                AF.Exp, bias=cb_t[:, jt:jt + 1], scale=SCALE,
                            )
                        if jt == 0:
                            nc.vector.tensor_copy(sacc[:], expT[0][:])
                        else:
                            nc.vector.tensor_tensor(
                                out=sacc[:], in0=sacc[:], in1=expT[jt][:], op=Alu.add,
                            )

                # ---- numerator + scale + store ----
                # Denominator matmuls are emitted after it=0's numerator
                # group: they depend on the DVE sum chain (exp jt=15 ->
                # sacc -> sume_bf) which finishes ~5us after the last
                # scores matmul; emitting them first would stall the PE.
                with tc.tile_pool(name="pnum", bufs=4, space="PSUM") as pnum:
                    nc.vector.tensor_copy(sume_bf[:], sacc[:])
                    for it in range(NT):
                        ps = [pnum.tile([P, 512], f32, tag="ps_n", name=f"psn{_i}") for _i in range(2)]
                        for jt in range(NT):
                            for ech in range(2):
                                nc.tensor.matmul(
                                    ps[ech][:],
                                    lhsT=expT[jt][:, it * P:(it + 1) * P],
                                    rhs=vt[:, jt, ech * 512:(ech + 1) * 512],
                                    start=(jt == 0),
                                    stop=(jt == NT - 1),
                                )
                        if it == 0:
                            pd = pnum.tile([P, 16], f32, tag="pd", bufs=1)
                            for dt in range(NT):
                                nc.tensor.matmul(
                                    pd[:, dt:dt + 1],
                                    lhsT=sume_bf[:, dt * P:(dt + 1) * P],
                                    rhs=ones_t[:],
                                    start=True, stop=True,
                                )
                            nc.vector.reciprocal(rdent[:], pd[:])
                        osb = attn.tile([P, E], f32, tag="osb", bufs=3)
                        for ech in range(2):
                            esl = slice(ech * 512, (ech + 1) * 512)
                            nc.scalar.activation(
                                osb[:, esl], ps[ech][:], AF.Copy,
                                scale=rdent[:, it:it + 1],
                            )
                            nc.sync.dma_start(out_r[it][:, esl], osb[:, esl])
    nc.compile()
    return nc


def get_nc():
    if "nc" not in _CACHE:
        _CACHE["nc"] = _build()
    return _CACHE["nc"]


def prepare_in_maps(x, W_qkv, b_qkv):
    bf = ml_dtypes.bfloat16
    x = np.asarray(x, dtype=np.float32)
    W = np.asarray(W_qkv, dtype=np.float32)
    b = np.asarray(b_qkv, dtype=np.float32)
    assert x.shape == (8, N, E) and W.shape == (3 * E, E) and b.shape == (3 * E,)
    xT = np.ascontiguousarray(np.transpose(x, (0, 2, 1))).astype(bf)  # [8, E, N]
    # fused QK weight: scores depend on Wq, Wk only through M = Wq^T Wk
    m = np.ascontiguousarray(W[:E].T @ W[E:2 * E]).astype(bf)         # [e1, e2]
    wv = np.ascontiguousarray(W[2 * E:].T).astype(bf)                 # [e_in, e_out]
    bv = np.ascontiguousarray(np.broadcast_to(b[2 * E:], (P, E)))     # [P, E]
    # per-key score bias c_j = x_j . (Wk^T bq), folded into exp bias
    m1 = W[E:2 * E].T @ b[:E]                                         # [E]
    cb = SCALE * (x @ m1)                                             # [8, N]
    cb = np.ascontiguousarray(cb.reshape(8, 16, P).transpose(0, 2, 1)).astype(np.float32)
    return [{"xT": xT[i], "m": m, "wv": wv,
             "cb": cb[i], "bv": bv} for i in range(8)]


def kernel(x, W_qkv, b_qkv):
    from concourse.bass_utils import run_bass_kernel_spmd

    nc = get_nc()
    in_maps = prepare_in_maps(x, W_qkv, b_qkv)
    res = run_bass_kernel_spmd(nc, in_maps, core_ids=list(range(8)))
    return np.stack([res.results[i]["out"] for i in range(8)], axis=0)
